# revision 3
# baseline (speedup 1.0000x reference)
"""Trainium2 Bass kernel: BasicMultiheadAttention (B=2, S=2048, D=1024, H=16).

Sharding: tensor-parallel over heads. Core c owns heads (2c, 2c+1) for both
batches; computes Q^T/K^T/V^T projections, attention in scores^T layout
(softmax exp on ACT, denominator via ones-augmented V in the PV matmul),
normalized ctx^T; per-query-chunk AllGather of ctx^T across the 8 cores; then
column-sharded output projection (+bias) per core.

v2 schedule: the whole kernel is one software-pipelined stream of 64 kt-pair
"groups" (2 batches x 4 query chunks x 8 pairs). Per group: scores pairs +
exp for kts (2g, 2g+1), then pinned filler work (QKV projection chunks,
V-transposes, output projections, denominator broadcasts), then the PV pairs
of group g-1 (trailing one group so the tensor engine never waits on exp).
ACT does exp only (biases on DVE); the denominator is extracted with a
per-head PSUM->SBUF cast, a row reciprocal, and a single K=1 ones-broadcast
matmul per head instead of transpose round-trips.
"""

import numpy as np

B, S, D, H = 2, 2048, 1024, 16
DH = D // H  # 64
NCORES = 8
HPC = H // NCORES  # heads per core = 2
SQ = B * S  # 4096 tokens
NKT = D // 128  # 8 contraction k-tiles over D
KT_S = S // 128  # 16 key tiles per batch
QC_S = S // 512  # 4 query chunks of 512 per batch

_CACHE = {}


def _ensure_axon_hooks():
    """This image's antenv lacks axon_hooks; bass_utils imports it when
    trace=True under axon. Register an equivalent stub backed by the boot
    helper so NTFF profiling works (or degrades gracefully)."""
    import sys
    import types
    try:
        import antenv.axon_hooks  # noqa: F401
        return
    except ImportError:
        pass
    try:
        import antenv
        hook = [None]
        try:
            from trn_agent_boot.trn_boot import _ntff_profile_via_ctypes
            hook[0] = _ntff_profile_via_ctypes("/opt/axon/libaxon_pjrt.so")
        except Exception:
            hook[0] = None
        mod = types.ModuleType("antenv.axon_hooks")
        mod.get_axon_ntff_profile_hook = lambda: hook[0]
        mod.set_axon_ntff_profile_hook = lambda h: hook.__setitem__(0, h)
        sys.modules["antenv.axon_hooks"] = mod
        antenv.axon_hooks = mod
    except Exception:
        pass


_ensure_axon_hooks()


def _build_kernel():
    import concourse.bass as bass  # noqa: F401
    import concourse.mybir as mybir
    import concourse.tile as tile
    from concourse import bacc
    from concourse.masks import make_identity

    f16 = mybir.dt.float16
    f32 = mybir.dt.float32
    AF = mybir.ActivationFunctionType

    nc = bacc.Bacc(None, num_devices=NCORES)

    # ---- I/O ----
    xT = nc.dram_tensor("xT", [D, SQ], f16, kind="ExternalInput")
    wpack = nc.dram_tensor("wpack", [128, 4 * D], f16, kind="ExternalInput")
    bpack = nc.dram_tensor("bpack", [128, 4], f32, kind="ExternalInput")
    yT = nc.dram_tensor("yT", [128, SQ], f32, kind="ExternalOutput")

    with tile.TileContext(nc) as tc:
        with (
            tc.tile_pool(name="const", bufs=1) as const,
            tc.tile_pool(name="psA", bufs=2, space="PSUM") as psA,
            tc.tile_pool(name="psSc", bufs=2, space="PSUM") as psSc,
            tc.tile_pool(name="psCtx", bufs=1, space="PSUM") as psCtx,
            tc.tile_pool(name="pP", bufs=6) as pP,
            tc.tile_pool(name="pDen", bufs=4) as pDen,
            tc.tile_pool(name="pOut", bufs=2) as pOut,
            tc.tile_pool(name="dram", bufs=1, space="DRAM") as dram,
        ):
            # ---- constants / weights (K first: attention needs K earliest) ----
            wpack_sb = const.tile([128, 4 * D], f16)
            wq_sb = wpack_sb[:, 0 * D:1 * D]
            wk_sb = wpack_sb[:, 1 * D:2 * D]
            wv_sb = wpack_sb[:, 2 * D:3 * D]
            wo_sb = wpack_sb[:, 3 * D:4 * D]
            nc.sync.dma_start(wk_sb, wpack[:, 1 * D:2 * D])
            nc.sync.dma_start(wq_sb, wpack[:, 0 * D:1 * D])
            nc.sync.dma_start(wv_sb, wpack[:, 2 * D:3 * D])
            bpack_sb = const.tile([128, 4], f32)
            nc.sync.dma_start(bpack_sb[:], bpack[:, :])
            bq_sb = bpack_sb[:, 0:1]
            bk_sb = bpack_sb[:, 1:2]
            bvt_sb = bpack_sb[:, 2:3]
            bo_sb = bpack_sb[:, 3:4]

            ident = const.tile([128, 128], f16)
            make_identity(nc, ident)
            ones1_64 = const.tile([1, 64], f16)
            nc.vector.memset(ones1_64[:], 1.0)

            # x in per-(batch,chunk,kt) pieces so QKV can start early
            xt_sb = const.tile([128, NKT * SQ], f16)
            for b in range(B):
                for c in range(QC_S):
                    t0 = b * S + c * 512
                    for kt in range(NKT):
                        nc.sync.dma_start(
                            xt_sb[:, kt * SQ + t0: kt * SQ + t0 + 512],
                            xT[kt * 128:(kt + 1) * 128, t0:t0 + 512],
                        )
            nc.sync.dma_start(wo_sb, wpack[:, 3 * D:4 * D])

            qT_sb = const.tile([128, SQ], f16)
            kT_sb = const.tile([128, SQ], f16)
            vT_sb = const.tile([128, SQ], f16)
            # V with ones column: per (b, head, key-tile) a [128, 65] region
            NREG = B * HPC * KT_S  # 64 regions
            vaug_sb = const.tile([128, NREG * 65], f16)
            ones_cols = vaug_sb.rearrange("p (r c) -> p r c", c=65)[:, :, 64:65]
            nc.vector.memset(ones_cols, 1.0)

            ctxT_sb = const.tile([128, SQ], f16)

            # warmup collective: absorb the first-trigger ncfw init delay early
            wu_loc = dram.tile([128, 2], f16, name="wu_loc")
            nc.sync.dma_start(wu_loc[:], ctxT_sb[:, 0:2])
            wu_g = dram.tile([NCORES * 128, 2], f16, addr_space="Shared", name="wu_g")
            nc.gpsimd.collective_compute(
                "AllGather", mybir.AluOpType.bypass,
                replica_groups=[list(range(NCORES))],
                ins=[wu_loc.opt()], outs=[wu_g.opt()],
            )

            # ---- helper emitters ----
            def proj_half(b, c, w_sb, state, half):
                """half 0: alloc psum + 4 accum matmuls; half 1: 4 matmuls."""
                tok0 = b * S + c * 512
                if half == 0:
                    state["ps"] = psA.tile(
                        [128, 512], f32, tag="a", name=f"pj_{b}_{c}_{id(w_sb) % 97}"
                    )
                ps = state["ps"]
                for kt in range(half * 4, half * 4 + 4):
                    nc.tensor.matmul(
                        ps[:],
                        lhsT=w_sb[:, kt * 128:(kt + 1) * 128],
                        rhs=xt_sb[:, kt * SQ + tok0: kt * SQ + tok0 + 512],
                        start=(kt == 0),
                        stop=(kt == NKT - 1),
                    )

            def proj_bias(b, c, b_sb, dst, state):
                tok0 = b * S + c * 512
                nc.vector.tensor_scalar_add(
                    dst[:, tok0:tok0 + 512], state["ps"][:], b_sb
                )

            def emit_proj(b, c, w_sb, b_sb, dst):
                """Whole projection chunk (for the un-overlapped preamble)."""
                st = {}
                proj_half(b, c, w_sb, st, 0)
                proj_half(b, c, w_sb, st, 1)
                proj_bias(b, c, b_sb, dst, st)

            def emit_vtr(b, c):
                """Transpose V^T chunk into vaug token-major regions."""
                vtr = psA.tile([128, 512], f16, tag="a", name=f"vtr_{b}_{c}")
                for t in range(4):
                    tok0 = b * S + c * 512 + t * 128
                    nc.tensor.transpose(
                        vtr[:, t * 128:(t + 1) * 128],
                        vT_sb[:, tok0:tok0 + 128], ident[:],
                    )
                for t in range(4):
                    tt = c * 4 + t
                    for h in range(HPC):
                        r = (b * HPC + h) * KT_S + tt
                        nc.vector.tensor_copy(
                            vaug_sb[:, r * 65: r * 65 + 64],
                            vtr[:, t * 128 + h * 64: t * 128 + (h + 1) * 64],
                        )

            def outproj_a(b, qc, cg, state):
                cgts = []
                for kt in range(NKT):
                    cgt = pOut.tile([128, 512], f16, tag=f"cg{kt}",
                                    name=f"cg_{b}_{qc}_{kt}")
                    nc.sync.dma_start(cgt[:], cg[kt * 128:(kt + 1) * 128, :])
                    cgts.append(cgt)
                state["cgts"] = cgts
                po = psA.tile([128, 512], f32, tag="a", name=f"opp_{b}_{qc}")
                state["po"] = po
                for kt in range(4):
                    nc.tensor.matmul(
                        po[:], lhsT=wo_sb[:, kt * 128:(kt + 1) * 128],
                        rhs=cgts[kt][:], start=(kt == 0), stop=False,
                    )

            def outproj_b(b, qc, q0, state):
                po = state["po"]
                for kt in range(4, NKT):
                    nc.tensor.matmul(
                        po[:], lhsT=wo_sb[:, kt * 128:(kt + 1) * 128],
                        rhs=state["cgts"][kt][:], start=False,
                        stop=(kt == NKT - 1),
                    )
                out_sb = pOut.tile([128, 512], f32, tag="os", name=f"os_{b}_{qc}")
                nc.vector.tensor_scalar_add(out_sb[:], po[:], bo_sb)
                nc.sync.dma_start(yT[:, q0:q0 + 512], out_sb[:])

            # ---- build the global group schedule ----
            # groups: (b, qc, g) with g in 0..7 covering kt = 2g, 2g+1
            groups = []
            for b in range(B):
                for qc in range(QC_S):
                    for g in range(8):
                        groups.append((b, qc, g))

            # pinned fillers per global group index
            fillers = {i: [] for i in range(len(groups))}

            def pin(agi, g, fns):
                fillers[agi * 8 + g].extend(fns)

            def mk_proj_fns(b, c, w_sb, b_sb, dst):
                st = {}
                return [
                    lambda: proj_half(b, c, w_sb, st, 0),
                    lambda: (proj_half(b, c, w_sb, st, 1),
                             proj_bias(b, c, b_sb, dst, st)),
                ]

            def mk_v_fns(b, c):
                st = {}
                return [
                    lambda: proj_half(b, c, wv_sb, st, 0),
                    lambda: (proj_half(b, c, wv_sb, st, 1),
                             proj_bias(b, c, bvt_sb, vT_sb, st)),
                    lambda: emit_vtr(b, c),
                ]

            outproj_fns = {}  # (b, qc) -> [fnA, fnB]; filled when gather emitted

            def mk_outproj_ref(b, qc):
                return [
                    lambda: outproj_fns[(b, qc)][0](),
                    lambda: outproj_fns[(b, qc)][1](),
                ]

            # AG(b0,qc0): feed K/V chunks 1-3 (chunk0 in preamble), next Q.
            # Constraint: K chunk c complete by group 2c-1 (scores of kt=4c
            # run at group 2c, emitted before that group's fillers); V chunk c
            # (incl. vtr) by group 2c+1 (its first PV trails there).
            kf = mk_proj_fns(0, 1, wk_sb, bk_sb, kT_sb)
            vf = mk_v_fns(0, 0)
            pin(0, 0, [kf[0], vf[0]])
            pin(0, 1, [kf[1], vf[1], vf[2]])
            kf = mk_proj_fns(0, 2, wk_sb, bk_sb, kT_sb)
            vf = mk_v_fns(0, 1)
            pin(0, 2, [kf[0], vf[0]])
            pin(0, 3, [kf[1], vf[1], vf[2]])
            kf = mk_proj_fns(0, 3, wk_sb, bk_sb, kT_sb)
            vf = mk_v_fns(0, 2)
            pin(0, 4, [kf[0], vf[0]])
            pin(0, 5, [kf[1], vf[1], vf[2]])
            qf = mk_proj_fns(0, 1, wq_sb, bq_sb, qT_sb)
            vf = mk_v_fns(0, 3)
            pin(0, 6, [qf[0], vf[0]])
            pin(0, 7, [qf[1], vf[1], vf[2]])

            # AG(b0,qc1): b1 K/V chunk0, outproj(b0,qc0), Q(b0,qc2)
            def pin_std(agi, kv_b, kv_c, op_bqc, q_bqc):
                if kv_b is not None:
                    kf = mk_proj_fns(kv_b, kv_c, wk_sb, bk_sb, kT_sb)
                    pin(agi, 0, [kf[0]])
                    pin(agi, 1, [kf[1]])
                    vf = mk_v_fns(kv_b, kv_c)
                    pin(agi, 2, [vf[0]])
                    pin(agi, 3, [vf[1]])
                    pin(agi, 4, [vf[2]])
                if op_bqc is not None:
                    opf = mk_outproj_ref(*op_bqc)
                    pin(agi, 5, [opf[0]])
                    pin(agi, 6, [opf[1]])
                if q_bqc is not None:
                    qf = mk_proj_fns(q_bqc[0], q_bqc[1], wq_sb, bq_sb, qT_sb)
                    pin(agi, 6, [qf[0]])
                    pin(agi, 7, [qf[1]])

            pin_std(1, 1, 0, (0, 0), (0, 2))
            pin_std(2, 1, 1, (0, 1), (0, 3))
            pin_std(3, 1, 2, (0, 2), (1, 0))
            pin_std(4, 1, 3, (0, 3), (1, 1))
            pin_std(5, None, None, (1, 0), (1, 2))
            pin_std(6, None, None, (1, 1), (1, 3))
            pin_std(7, None, None, (1, 2), None)

            # ---- preamble: first K and Q chunks (attention prerequisites) ----
            emit_proj(0, 0, wk_sb, bk_sb, kT_sb)
            emit_proj(0, 0, wq_sb, bq_sb, qT_sb)

            # ---- main software-pipelined stream ----
            ag_state = {}  # (b, qc) -> {"ctx":..., "ps": [p tiles by kt]}

            def emit_scores(b, qc, kt):
                st = ag_state.setdefault((b, qc), {"p": {}})
                q0 = b * S + qc * 512
                k0 = b * S + kt * 128
                sc = psSc.tile([128, 1024], f32, tag="sc", name=f"sc_{b}_{qc}_{kt}")
                for h in range(HPC):
                    nc.tensor.matmul(
                        sc[:, h * 512:(h + 1) * 512],
                        lhsT=kT_sb[h * 64:(h + 1) * 64, k0:k0 + 128],
                        rhs=qT_sb[h * 64:(h + 1) * 64, q0:q0 + 512],
                        start=True, stop=True,
                        tile_position=(h * 64, 0),
                    )
                p = pP.tile([128, 1024], f16, tag="p", name=f"p_{b}_{qc}_{kt}")
                nc.scalar.activation(p[:], sc[:], AF.Exp, scale=0.125)
                st["p"][kt] = p

            def emit_pv(b, qc, kt):
                st = ag_state[(b, qc)]
                if kt == 0:
                    st["ctx"] = psCtx.tile(
                        [128, 1024], f32, tag="ctx", name=f"ctx_{b}_{qc}"
                    )
                ctx = st["ctx"]
                p = st["p"].pop(kt)
                for h in range(HPC):
                    r = (b * HPC + h) * KT_S + kt
                    nc.tensor.matmul(
                        ctx[0:65, h * 512:(h + 1) * 512],
                        lhsT=vaug_sb[:, r * 65:(r + 1) * 65],
                        rhs=p[:, h * 512:(h + 1) * 512],
                        start=(kt == 0),
                        stop=(kt == KT_S - 1),
                    )

            def emit_boundary(b, qc):
                """ctx -> normalized ctxT + AllGather; registers outproj fns."""
                st = ag_state[(b, qc)]
                ctx = st["ctx"]
                q0 = b * S + qc * 512
                ctxus = []
                for h in range(HPC):
                    ctxu = pDen.tile([65, 512], f16, tag="cu",
                                     name=f"cu_{b}_{qc}_{h}")
                    nc.vector.tensor_copy(ctxu[:], ctx[0:65, h * 512:(h + 1) * 512])
                    ctxus.append(ctxu)
                recs = []
                for h in range(HPC):
                    rec = pDen.tile([1, 512], f16, tag="rec",
                                    name=f"rec_{b}_{qc}_{h}")
                    with nc.allow_low_precision(reason="softmax denom recip f16"):
                        nc.vector.reciprocal(rec[:], ctxus[h][64:65, :])
                    recs.append(rec)
                for h in range(HPC):
                    rps = psA.tile([64, 512], f32, tag="a",
                                   name=f"rps_{b}_{qc}_{h}")
                    nc.tensor.matmul(
                        rps[:], lhsT=ones1_64[:], rhs=recs[h][:],
                        start=True, stop=True,
                    )
                    nc.vector.tensor_mul(
                        ctxT_sb[h * 64:(h + 1) * 64, q0:q0 + 512],
                        ctxus[h][0:64, :], rps[:],
                    )
                ctx_loc = dram.tile([128, 512], f16, name=f"ctx_loc_{b}_{qc}")
                nc.sync.dma_start(ctx_loc[:], ctxT_sb[:, q0:q0 + 512])
                cg = dram.tile([NCORES * 128, 512], f16, addr_space="Shared",
                               name=f"ctx_gath_{b}_{qc}")
                nc.gpsimd.collective_compute(
                    "AllGather", mybir.AluOpType.bypass,
                    replica_groups=[list(range(NCORES))],
                    ins=[ctx_loc.opt()], outs=[cg.opt()],
                )
                opst = {}
                outproj_fns[(b, qc)] = [
                    lambda: outproj_a(b, qc, cg, opst),
                    lambda: outproj_b(b, qc, q0, opst),
                ]

            n = len(groups)
            for i in range(n):
                b, qc, g = groups[i]
                # scores + exp for this group's two kts
                emit_scores(b, qc, 2 * g)
                emit_scores(b, qc, 2 * g + 1)
                # pinned fillers (QKV chunks, outproj, denominators ride here)
                for fn in fillers[i]:
                    fn()
                # trailing PV of the previous group
                if i > 0:
                    pb, pqc, pg = groups[i - 1]
                    emit_pv(pb, pqc, 2 * pg)
                    emit_pv(pb, pqc, 2 * pg + 1)
                    if pg == 7:
                        emit_boundary(pb, pqc)
            # final trailing PV + boundary + last outproj
            b, qc, g = groups[-1]
            emit_pv(b, qc, 2 * g)
            emit_pv(b, qc, 2 * g + 1)
            emit_boundary(b, qc)
            outproj_fns[(b, qc)][0]()
            outproj_fns[(b, qc)][1]()

    nc.finalize()
    return nc


def kernel(x, Wq, Wk, Wv, bq, bk, bv, Wo, bo):
    from concourse.bass_utils import run_bass_kernel_spmd

    if "nc" not in _CACHE:
        _CACHE["nc"] = _build_kernel()
    nc = _CACHE["nc"]

    # host-side prep
    xTh = np.ascontiguousarray(
        x.astype(np.float32).transpose(2, 0, 1).reshape(D, SQ)
    ).astype(np.float16)

    def pack_w(Wslice):
        # [D, 128] -> [128, D] kt-major: out[p, kt*128+m] = Wslice[kt*128+p, m]
        return np.ascontiguousarray(
            Wslice.reshape(NKT, 128, 128).transpose(1, 0, 2).reshape(128, D)
        ).astype(np.float16)

    in_maps = []
    for c in range(NCORES):
        hA, hB = HPC * c, HPC * c + 1
        wq_c = pack_w(np.concatenate([Wq[hA], Wq[hB]], axis=1))
        wk_c = pack_w(np.concatenate([Wk[hA], Wk[hB]], axis=1))
        wv_c = pack_w(np.concatenate([Wv[hA], Wv[hB]], axis=1))
        wo_c = pack_w(Wo[:, 128 * c:128 * (c + 1)])
        wpack_c = np.ascontiguousarray(
            np.concatenate([wq_c, wk_c, wv_c, wo_c], axis=1))
        bq_c = np.concatenate([bq[hA], bq[hB]]).reshape(128, 1)
        bk_c = np.concatenate([bk[hA], bk[hB]]).reshape(128, 1)
        bv_c = np.concatenate([bv[hA], bv[hB]]).reshape(128, 1)
        bo_c = bo[128 * c:128 * (c + 1)].reshape(128, 1)
        bpack_c = np.ascontiguousarray(
            np.concatenate([bq_c, bk_c, bv_c, bo_c], axis=1)).astype(np.float32)
        in_maps.append({"xT": xTh, "wpack": wpack_c, "bpack": bpack_c})

    res = run_bass_kernel_spmd(nc, in_maps, core_ids=list(range(NCORES)))
    _CACHE["last_result"] = res
    # assemble: core c's yT [128, SQ] are output columns 128c..128c+127 (transposed)
    out = np.empty((B, S, D), dtype=np.float32)
    for c in range(NCORES):
        yt = res.results[c]["yT"]  # [128, SQ]
        out[:, :, 128 * c:128 * (c + 1)] = (
            yt.reshape(128, B, S).transpose(1, 2, 0)
        )
    return out


# revision 6
# speedup vs baseline: 1.1841x; 1.1841x over previous
"""Trainium2 Bass kernel: BasicMultiheadAttention (B=2, S=2048, D=1024, H=16).

Sharding: tensor-parallel over heads. Core c owns heads (2c, 2c+1) for both
batches; computes Q^T/K^T/V^T projections, attention in scores^T layout
(softmax exp on ACT, denominator via ones-augmented V in the PV matmul),
normalized ctx^T; per-query-chunk AllGather of ctx^T across the 8 cores; then
column-sharded output projection (+bias) per core.

v2 schedule: the whole kernel is one software-pipelined stream of 64 kt-pair
"groups" (2 batches x 4 query chunks x 8 pairs). Per group: scores pairs +
exp for kts (2g, 2g+1), then pinned filler work (QKV projection chunks,
V-transposes, output projections, denominator broadcasts), then the PV pairs
of group g-1 (trailing one group so the tensor engine never waits on exp).
ACT does exp only (biases on DVE); the denominator is extracted with a
per-head PSUM->SBUF cast, a row reciprocal, and a single K=1 ones-broadcast
matmul per head instead of transpose round-trips.
"""

import numpy as np

B, S, D, H = 2, 2048, 1024, 16
DH = D // H  # 64
NCORES = 8
HPC = H // NCORES  # heads per core = 2
SQ = B * S  # 4096 tokens
NKT = D // 128  # 8 contraction k-tiles over D
KT_S = S // 128  # 16 key tiles per batch
QC_S = S // 512  # 4 query chunks of 512 per batch

_CACHE = {}


def _ensure_axon_hooks():
    """This image's antenv lacks axon_hooks; bass_utils imports it when
    trace=True under axon. Register an equivalent stub backed by the boot
    helper so NTFF profiling works (or degrades gracefully)."""
    import sys
    import types
    try:
        import antenv.axon_hooks  # noqa: F401
        return
    except ImportError:
        pass
    try:
        import antenv
        hook = [None]
        try:
            from trn_agent_boot.trn_boot import _ntff_profile_via_ctypes
            hook[0] = _ntff_profile_via_ctypes("/opt/axon/libaxon_pjrt.so")
        except Exception:
            hook[0] = None
        mod = types.ModuleType("antenv.axon_hooks")
        mod.get_axon_ntff_profile_hook = lambda: hook[0]
        mod.set_axon_ntff_profile_hook = lambda h: hook.__setitem__(0, h)
        sys.modules["antenv.axon_hooks"] = mod
        antenv.axon_hooks = mod
    except Exception:
        pass


_ensure_axon_hooks()


def _build_kernel():
    import concourse.bass as bass  # noqa: F401
    import concourse.mybir as mybir
    import concourse.tile as tile
    from concourse import bacc
    from concourse.masks import make_identity

    f16 = mybir.dt.float16
    f32 = mybir.dt.float32
    AF = mybir.ActivationFunctionType

    nc = bacc.Bacc(None, num_devices=NCORES)

    # ---- I/O ----
    xT = nc.dram_tensor("xT", [D, SQ], f16, kind="ExternalInput")
    wpack = nc.dram_tensor("wpack", [128, 4 * D], f16, kind="ExternalInput")
    bpack = nc.dram_tensor("bpack", [128, 4], f32, kind="ExternalInput")
    yT = nc.dram_tensor("yT", [128, SQ], f32, kind="ExternalOutput")

    with tile.TileContext(nc) as tc:
        with (
            tc.tile_pool(name="const", bufs=1) as const,
            tc.tile_pool(name="psA", bufs=2, space="PSUM") as psA,
            tc.tile_pool(name="psSc", bufs=2, space="PSUM") as psSc,
            tc.tile_pool(name="psCtx", bufs=1, space="PSUM") as psCtx,
            tc.tile_pool(name="pP", bufs=6) as pP,
            tc.tile_pool(name="pDen", bufs=4) as pDen,
            tc.tile_pool(name="pOut", bufs=2) as pOut,
            tc.tile_pool(name="dram", bufs=1, space="DRAM") as dram,
        ):
            # warmup collective first: absorb the ~60us first-trigger ncfw
            # init before the real gathers queue up behind it
            wu_loc = dram.tile([128, 2], f16, name="wu_loc")
            wu_src = const.tile([1, 2], f16)
            nc.vector.memset(wu_src[:], 0.0)
            nc.sync.dma_start(wu_loc[0:1, :], wu_src[:])
            wu_g = dram.tile([NCORES * 128, 2], f16, addr_space="Shared", name="wu_g")
            nc.gpsimd.collective_compute(
                "AllGather", mybir.AluOpType.bypass,
                replica_groups=[list(range(NCORES))],
                ins=[wu_loc.opt()], outs=[wu_g.opt()],
            )

            # ---- constants / weights (K first: attention needs K earliest) ----
            wpack_sb = const.tile([128, 4 * D], f16)
            wq_sb = wpack_sb[:, 0 * D:1 * D]
            wk_sb = wpack_sb[:, 1 * D:2 * D]
            wv_sb = wpack_sb[:, 2 * D:3 * D]
            wo_sb = wpack_sb[:, 3 * D:4 * D]
            nc.sync.dma_start(wk_sb, wpack[:, 1 * D:2 * D])
            nc.sync.dma_start(wq_sb, wpack[:, 0 * D:1 * D])
            nc.sync.dma_start(wv_sb, wpack[:, 2 * D:3 * D])
            bpack_sb = const.tile([128, 4], f32)
            nc.sync.dma_start(bpack_sb[:], bpack[:, :])
            bq_sb = bpack_sb[:, 0:1]
            bk_sb = bpack_sb[:, 1:2]
            bvt_sb = bpack_sb[:, 2:3]
            bo_sb = bpack_sb[:, 3:4]

            ident = const.tile([128, 128], f16)
            make_identity(nc, ident)
            ones1_64 = const.tile([1, 64], f16)
            nc.vector.memset(ones1_64[:], 1.0)

            # x in per-(batch,chunk,kt) pieces so QKV can start early
            xt_sb = const.tile([128, NKT * SQ], f16)
            for b in range(B):
                for c in range(QC_S):
                    t0 = b * S + c * 512
                    for kt in range(NKT):
                        nc.sync.dma_start(
                            xt_sb[:, kt * SQ + t0: kt * SQ + t0 + 512],
                            xT[kt * 128:(kt + 1) * 128, t0:t0 + 512],
                        )
            nc.sync.dma_start(wo_sb, wpack[:, 3 * D:4 * D])

            qT_sb = const.tile([128, SQ], f16)
            kT_sb = const.tile([128, SQ], f16)
            vT_sb = const.tile([128, SQ], f16)
            # V with ones column: per (b, head, key-tile) a [128, 65] region
            NREG = B * HPC * KT_S  # 64 regions
            vaug_sb = const.tile([128, NREG * 65], f16)
            ones_cols = vaug_sb.rearrange("p (r c) -> p r c", c=65)[:, :, 64:65]
            nc.vector.memset(ones_cols, 1.0)

            ctxT_sb = const.tile([128, SQ], f16)

            # ---- helper emitters ----
            def proj_half(b, c, w_sb, state, half):
                """half 0: alloc psum + 4 accum matmuls; half 1: 4 matmuls."""
                tok0 = b * S + c * 512
                if half == 0:
                    state["ps"] = psA.tile(
                        [128, 512], f32, tag="a", name=f"pj_{b}_{c}_{id(w_sb) % 97}"
                    )
                ps = state["ps"]
                for kt in range(half * 4, half * 4 + 4):
                    nc.tensor.matmul(
                        ps[:],
                        lhsT=w_sb[:, kt * 128:(kt + 1) * 128],
                        rhs=xt_sb[:, kt * SQ + tok0: kt * SQ + tok0 + 512],
                        start=(kt == 0),
                        stop=(kt == NKT - 1),
                    )

            def proj_bias(b, c, b_sb, dst, state):
                tok0 = b * S + c * 512
                nc.vector.tensor_scalar_add(
                    dst[:, tok0:tok0 + 512], state["ps"][:], b_sb
                )

            def emit_proj(b, c, w_sb, b_sb, dst):
                """Whole projection chunk (for the un-overlapped preamble)."""
                st = {}
                proj_half(b, c, w_sb, st, 0)
                proj_half(b, c, w_sb, st, 1)
                proj_bias(b, c, b_sb, dst, st)

            def emit_vtr(b, c):
                """Transpose V^T chunk into vaug token-major regions."""
                vtr = psA.tile([128, 512], f16, tag="a", name=f"vtr_{b}_{c}")
                for t in range(4):
                    tok0 = b * S + c * 512 + t * 128
                    nc.tensor.transpose(
                        vtr[:, t * 128:(t + 1) * 128],
                        vT_sb[:, tok0:tok0 + 128], ident[:],
                    )
                for t in range(4):
                    tt = c * 4 + t
                    for h in range(HPC):
                        r = (b * HPC + h) * KT_S + tt
                        nc.vector.tensor_copy(
                            vaug_sb[:, r * 65: r * 65 + 64],
                            vtr[:, t * 128 + h * 64: t * 128 + (h + 1) * 64],
                        )

            def outproj_a(b, qc, cg, state):
                cgts = []
                for kt in range(NKT):
                    cgt = pOut.tile([128, 512], f16, tag=f"cg{kt}",
                                    name=f"cg_{b}_{qc}_{kt}")
                    nc.sync.dma_start(cgt[:], cg[kt * 128:(kt + 1) * 128, :])
                    cgts.append(cgt)
                state["cgts"] = cgts
                po = psA.tile([128, 512], f32, tag="a", name=f"opp_{b}_{qc}")
                state["po"] = po
                for kt in range(4):
                    nc.tensor.matmul(
                        po[:], lhsT=wo_sb[:, kt * 128:(kt + 1) * 128],
                        rhs=cgts[kt][:], start=(kt == 0), stop=False,
                    )

            def outproj_b(b, qc, q0, state):
                po = state["po"]
                for kt in range(4, NKT):
                    nc.tensor.matmul(
                        po[:], lhsT=wo_sb[:, kt * 128:(kt + 1) * 128],
                        rhs=state["cgts"][kt][:], start=False,
                        stop=(kt == NKT - 1),
                    )
                out_sb = pOut.tile([128, 512], f32, tag="os", name=f"os_{b}_{qc}")
                nc.vector.tensor_scalar_add(out_sb[:], po[:], bo_sb)
                nc.sync.dma_start(yT[:, q0:q0 + 512], out_sb[:])

            # ---- build the global group schedule ----
            # groups: (b, qc, g) with g in 0..7 covering kt = 2g, 2g+1
            groups = []
            for b in range(B):
                for qc in range(QC_S):
                    for g in range(8):
                        groups.append((b, qc, g))

            # pinned fillers per global group index
            fillers = {i: [] for i in range(len(groups))}

            def pin(agi, g, fns):
                fillers[agi * 8 + g].extend(fns)

            def mk_proj_fns(b, c, w_sb, b_sb, dst):
                st = {}
                return [
                    lambda: proj_half(b, c, w_sb, st, 0),
                    lambda: (proj_half(b, c, w_sb, st, 1),
                             proj_bias(b, c, b_sb, dst, st)),
                ]

            def mk_v_fns(b, c):
                st = {}
                return [
                    lambda: proj_half(b, c, wv_sb, st, 0),
                    lambda: (proj_half(b, c, wv_sb, st, 1),
                             proj_bias(b, c, bvt_sb, vT_sb, st)),
                    lambda: emit_vtr(b, c),
                ]

            outproj_fns = {}  # (b, qc) -> [fnA, fnB]; filled when gather emitted

            def mk_outproj_ref(b, qc):
                return [
                    lambda: outproj_fns[(b, qc)][0](),
                    lambda: outproj_fns[(b, qc)][1](),
                ]

            # AG(b0,qc0): feed K/V chunks 1-3 (chunk0 in preamble), next Q.
            # Constraint: K chunk c complete by group 2c-1 (scores of kt=4c
            # run at group 2c, emitted before that group's fillers); V chunk c
            # (incl. vtr) by group 2c+1 (its first PV trails there).
            kf = mk_proj_fns(0, 1, wk_sb, bk_sb, kT_sb)
            vf = mk_v_fns(0, 0)
            pin(0, 0, [kf[0], vf[0]])
            pin(0, 1, [kf[1], vf[1], vf[2]])
            kf = mk_proj_fns(0, 2, wk_sb, bk_sb, kT_sb)
            vf = mk_v_fns(0, 1)
            pin(0, 2, [kf[0], vf[0]])
            pin(0, 3, [kf[1], vf[1], vf[2]])
            kf = mk_proj_fns(0, 3, wk_sb, bk_sb, kT_sb)
            vf = mk_v_fns(0, 2)
            pin(0, 4, [kf[0], vf[0]])
            pin(0, 5, [kf[1], vf[1], vf[2]])
            qf = mk_proj_fns(0, 1, wq_sb, bq_sb, qT_sb)
            vf = mk_v_fns(0, 3)
            pin(0, 6, [qf[0], vf[0]])
            pin(0, 7, [qf[1], vf[1], vf[2]])

            def pin_kv(agi, kv_b, kv_c):
                kf = mk_proj_fns(kv_b, kv_c, wk_sb, bk_sb, kT_sb)
                pin(agi, 0, [kf[0]])
                pin(agi, 1, [kf[1]])
                vf = mk_v_fns(kv_b, kv_c)
                pin(agi, 2, [vf[0]])
                pin(agi, 3, [vf[1]])
                pin(agi, 4, [vf[2]])

            def pin_q(agi, g, b, qc):
                qf = mk_proj_fns(b, qc, wq_sb, bq_sb, qT_sb)
                pin(agi, g, [qf[0]])
                pin(agi, g + 1, [qf[1]])

            def pin_op(agi, g, b, qc):
                opf = mk_outproj_ref(b, qc)
                pin(agi, g, [opf[0]])
                pin(agi, g + 1, [opf[1]])

            # b1 QKV spread over AGs 1-4; outprojs deferred ~3 AGs past their
            # gather (first gather queues behind the ~60us warmup collective)
            pin_kv(1, 1, 0)
            pin_q(1, 6, 0, 2)
            pin_kv(2, 1, 1)
            pin_q(2, 6, 0, 3)
            pin_kv(3, 1, 2)
            pin_q(3, 6, 1, 0)
            pin_kv(4, 1, 3)
            pin_op(4, 5, 0, 0)
            pin_q(4, 6, 1, 1)
            pin_op(5, 0, 0, 1)
            pin_op(5, 3, 0, 2)
            pin_q(5, 6, 1, 2)
            pin_op(6, 0, 0, 3)
            pin_op(6, 4, 1, 0)
            pin_q(6, 6, 1, 3)
            pin_op(7, 1, 1, 1)
            pin_op(7, 5, 1, 2)

            # ---- preamble: first K and Q chunks (attention prerequisites) ----
            emit_proj(0, 0, wk_sb, bk_sb, kT_sb)
            emit_proj(0, 0, wq_sb, bq_sb, qT_sb)

            # ---- main software-pipelined stream ----
            ag_state = {}  # (b, qc) -> {"ctx":..., "ps": [p tiles by kt]}

            def emit_scores(b, qc, kt):
                st = ag_state.setdefault((b, qc), {"p": {}})
                q0 = b * S + qc * 512
                k0 = b * S + kt * 128
                sc = psSc.tile([128, 1024], f32, tag="sc", name=f"sc_{b}_{qc}_{kt}")
                for h in range(HPC):
                    nc.tensor.matmul(
                        sc[:, h * 512:(h + 1) * 512],
                        lhsT=kT_sb[h * 64:(h + 1) * 64, k0:k0 + 128],
                        rhs=qT_sb[h * 64:(h + 1) * 64, q0:q0 + 512],
                        start=True, stop=True,
                        tile_position=(h * 64, 0),
                    )
                p = pP.tile([128, 1024], f16, tag="p", name=f"p_{b}_{qc}_{kt}")
                nc.scalar.activation(p[:], sc[:], AF.Exp, scale=0.125)
                st["p"][kt] = p

            def emit_pv(b, qc, kt):
                st = ag_state[(b, qc)]
                if kt == 0:
                    st["ctx"] = psCtx.tile(
                        [128, 1024], f32, tag="ctx", name=f"ctx_{b}_{qc}"
                    )
                ctx = st["ctx"]
                p = st["p"].pop(kt)
                for h in range(HPC):
                    r = (b * HPC + h) * KT_S + kt
                    nc.tensor.matmul(
                        ctx[0:65, h * 512:(h + 1) * 512],
                        lhsT=vaug_sb[:, r * 65:(r + 1) * 65],
                        rhs=p[:, h * 512:(h + 1) * 512],
                        start=(kt == 0),
                        stop=(kt == KT_S - 1),
                    )

            def emit_boundary(b, qc):
                """ctx -> normalized ctxT + AllGather; registers outproj fns."""
                st = ag_state[(b, qc)]
                ctx = st["ctx"]
                q0 = b * S + qc * 512
                ctxus = []
                for h in range(HPC):
                    ctxu = pDen.tile([65, 512], f16, tag="cu",
                                     name=f"cu_{b}_{qc}_{h}")
                    nc.vector.tensor_copy(ctxu[:], ctx[0:65, h * 512:(h + 1) * 512])
                    ctxus.append(ctxu)
                recs = []
                for h in range(HPC):
                    rec = pDen.tile([1, 512], f16, tag="rec",
                                    name=f"rec_{b}_{qc}_{h}")
                    with nc.allow_low_precision(reason="softmax denom recip f16"):
                        nc.vector.reciprocal(rec[:], ctxus[h][64:65, :])
                    recs.append(rec)
                for h in range(HPC):
                    rps = psA.tile([64, 512], f32, tag="a",
                                   name=f"rps_{b}_{qc}_{h}")
                    nc.tensor.matmul(
                        rps[:], lhsT=ones1_64[:], rhs=recs[h][:],
                        start=True, stop=True,
                    )
                    nc.vector.tensor_mul(
                        ctxT_sb[h * 64:(h + 1) * 64, q0:q0 + 512],
                        ctxus[h][0:64, :], rps[:],
                    )
                ctx_loc = dram.tile([128, 512], f16, name=f"ctx_loc_{b}_{qc}")
                nc.sync.dma_start(ctx_loc[:], ctxT_sb[:, q0:q0 + 512])
                cg = dram.tile([NCORES * 128, 512], f16, addr_space="Shared",
                               name=f"ctx_gath_{b}_{qc}")
                nc.gpsimd.collective_compute(
                    "AllGather", mybir.AluOpType.bypass,
                    replica_groups=[list(range(NCORES))],
                    ins=[ctx_loc.opt()], outs=[cg.opt()],
                )
                opst = {}
                outproj_fns[(b, qc)] = [
                    lambda: outproj_a(b, qc, cg, opst),
                    lambda: outproj_b(b, qc, q0, opst),
                ]

            n = len(groups)
            for i in range(n):
                b, qc, g = groups[i]
                # scores + exp for this group's two kts
                emit_scores(b, qc, 2 * g)
                emit_scores(b, qc, 2 * g + 1)
                # pinned fillers (QKV chunks, outproj, denominators ride here)
                for fn in fillers[i]:
                    fn()
                # trailing PV of the previous group
                if i > 0:
                    pb, pqc, pg = groups[i - 1]
                    emit_pv(pb, pqc, 2 * pg)
                    emit_pv(pb, pqc, 2 * pg + 1)
                    if pg == 7:
                        emit_boundary(pb, pqc)
            # final trailing PV + boundary + last outproj
            b, qc, g = groups[-1]
            emit_pv(b, qc, 2 * g)
            emit_pv(b, qc, 2 * g + 1)
            emit_boundary(b, qc)
            outproj_fns[(b, qc)][0]()
            outproj_fns[(b, qc)][1]()

    nc.finalize()
    return nc


def kernel(x, Wq, Wk, Wv, bq, bk, bv, Wo, bo):
    from concourse.bass_utils import run_bass_kernel_spmd

    if "nc" not in _CACHE:
        _CACHE["nc"] = _build_kernel()
    nc = _CACHE["nc"]

    # host-side prep
    xTh = np.ascontiguousarray(
        x.astype(np.float32).transpose(2, 0, 1).reshape(D, SQ)
    ).astype(np.float16)

    def pack_w(Wslice):
        # [D, 128] -> [128, D] kt-major: out[p, kt*128+m] = Wslice[kt*128+p, m]
        return np.ascontiguousarray(
            Wslice.reshape(NKT, 128, 128).transpose(1, 0, 2).reshape(128, D)
        ).astype(np.float16)

    in_maps = []
    for c in range(NCORES):
        hA, hB = HPC * c, HPC * c + 1
        wq_c = pack_w(np.concatenate([Wq[hA], Wq[hB]], axis=1))
        wk_c = pack_w(np.concatenate([Wk[hA], Wk[hB]], axis=1))
        wv_c = pack_w(np.concatenate([Wv[hA], Wv[hB]], axis=1))
        wo_c = pack_w(Wo[:, 128 * c:128 * (c + 1)])
        wpack_c = np.ascontiguousarray(
            np.concatenate([wq_c, wk_c, wv_c, wo_c], axis=1))
        bq_c = np.concatenate([bq[hA], bq[hB]]).reshape(128, 1)
        bk_c = np.concatenate([bk[hA], bk[hB]]).reshape(128, 1)
        bv_c = np.concatenate([bv[hA], bv[hB]]).reshape(128, 1)
        bo_c = bo[128 * c:128 * (c + 1)].reshape(128, 1)
        bpack_c = np.ascontiguousarray(
            np.concatenate([bq_c, bk_c, bv_c, bo_c], axis=1)).astype(np.float32)
        in_maps.append({"xT": xTh, "wpack": wpack_c, "bpack": bpack_c})

    res = run_bass_kernel_spmd(nc, in_maps, core_ids=list(range(NCORES)))
    _CACHE["last_result"] = res
    # assemble: core c's yT [128, SQ] are output columns 128c..128c+127 (transposed)
    out = np.empty((B, S, D), dtype=np.float32)
    for c in range(NCORES):
        yt = res.results[c]["yT"]  # [128, SQ]
        out[:, :, 128 * c:128 * (c + 1)] = (
            yt.reshape(128, B, S).transpose(1, 2, 0)
        )
    return out
